# revision 19
# baseline (speedup 1.0000x reference)
"""Adaptive-softmax logits kernel for trn2 (8 NeuronCores, SPMD).

Problem: out = concat([hidden @ head_w,
                       ((hidden @ down0) @ dec0) * m0,
                       ((hidden @ down1) @ dec1) * m1], axis=1)
with hidden [2048, 1024], head_w [1024, 2002], dec0 [1024, 8000],
dec1 [256, 40000]; m0/m1 are per-row cluster masks from `target`.

Sharding: vocab-parallel. Each core gets 1/8 of every output segment
(head padded 2002->2048 so each core takes 256 head + 1000 t0 + 5000 t1
columns). Three host-side (algebraic, exact) restructurings cut the
device work:

1. The t0 branch is folded (W0 = down0 @ dec0, so t0 = hidden @ W0),
   removing the redundant 1024x1024 down-projection from every core.
2. Batch rows are permuted so cluster-1 rows come first, then
   cluster-0 rows, then the rest (row permutation commutes with every
   per-row op; the host inverse-permutes the assembled output). Each
   tail decode then only runs over the batch tiles its cluster
   occupies (~13/16 for t1, ~4/16 for t0 at the 80%/16% cluster
   rates); all other tail logits are exact zeros the host fills in.
   The on-device loop structure is compiled per (tile-range) tuple and
   cached, so any input distribution still produces a correct kernel.
3. Everything is bf16 (PSUM accumulates fp32): same 1 cycle/row PE
   rate as fp32r, half the HBM/SBUF bytes, absmax rel err ~3e-3 vs
   the 2e-2 gate. Output is written bf16 and upcast on the host.

On-device layout: out[b, v] tiles, batch on partitions; lhsT = x^T
k-chunk [128, 128] stationary, rhs = W [128, <=512] moving. Row masks
(per batch row = per partition) are applied during PSUM->SBUF
eviction as per-partition scalar multiplies, spread across the
Activation, DVE and Pool engines. Output columns are ordered
[t0 | head | t1] so whatever subset of segments a 128-row tile
computes is one contiguous span -> a single dma_start per tile.
"""

import numpy as np
import ml_dtypes

import concourse.mybir as mybir
import concourse.tile as tile
from concourse import bacc
from concourse.bass_utils import run_bass_kernel_spmd

# Problem shapes (hardcoded per the grading contract).
B = 2048  # batch
H = 1024  # hidden
NCORES = 8
P = 128
KC = H // P  # 8 k-chunks for K=1024 contractions
HEAD = 2002
HEAD_PAD = 2048  # padded so each core gets 256 head columns
T0 = 8000  # cluster-0 decode width
T1 = 40000  # cluster-1 decode width
R1 = 256  # tail-1 down-projection width (down1 columns)
KC1 = R1 // P  # 2 k-chunks for the t1 decode contraction

# Per-core column counts.
HEAD_C = HEAD_PAD // NCORES  # 256
T0_C = T0 // NCORES  # 1000
T1_C = T1 // NCORES  # 5000
OUT_C = HEAD_C + T0_C + T1_C  # 6256
# On-device column layout: [t0 | head | t1] so any contiguous run of
# active segments is one DMA.
C_T0 = 0
C_HEAD = T0_C
C_T1 = T0_C + HEAD_C

NBT = B // P  # 16 batch tiles of 128 rows
VT = 500  # decode free-dim tile
T1_VT = T1_C // VT  # 10

BH = 512  # psum bank = 512 fp32
BH1 = 256  # h1-phase batch tile / hT DMA chunk
NBH1 = B // BH1  # 8

F32 = mybir.dt.float32
BF16 = mybir.dt.bfloat16

NP_BF16 = np.dtype(ml_dtypes.bfloat16)

_compiled = {}  # (n1t, t0lo, t0hi) -> nc


def _build(n1t, t0lo, t0hi):
    """n1t: # of 128-row tiles (from 0) computing t1; [t0lo, t0hi): tile
    range computing t0. All 16 tiles compute the head."""
    nc = bacc.Bacc(None)

    hT = nc.declare_dram_parameter("hT", [H, B], BF16, isOutput=False)
    wh = nc.declare_dram_parameter("wh", [H, HEAD_C], BF16, isOutput=False)
    w0 = nc.declare_dram_parameter("w0", [H, T0_C], BF16, isOutput=False)
    down1 = nc.declare_dram_parameter("down1", [H, R1], BF16, isOutput=False)
    d1 = nc.declare_dram_parameter("d1", [R1, T1_C], BF16, isOutput=False)
    m0c = nc.declare_dram_parameter("m0c", [P, NBT], F32, isOutput=False)
    m1c = nc.declare_dram_parameter("m1c", [P, NBT], F32, isOutput=False)
    out = nc.declare_dram_parameter("out", [B, OUT_C], BF16, isOutput=True)

    hT3 = hT.rearrange("(ko p) b -> p ko b", p=P)
    n1ch = -(-(n1t * P) // BH1) if n1t else 0  # hT chunks the h1 phase needs

    with tile.TileContext(nc) as tc:
        with (
            tc.tile_pool(name="consts", bufs=1) as consts,
            tc.tile_pool(name="opool", bufs=8) as opool,
            tc.tile_pool(name="psum", bufs=8, space="PSUM") as psum,
        ):
            # Input DMAs, in the order compute consumes them (DMA transfers
            # serialize, so this order sets when each tensor lands).
            down1_sb = consts.tile([P, KC, R1], BF16)
            dn3 = down1.rearrange("(ko p) m -> p ko m", p=P)
            # hT chunking: a small first chunk so the PE starts ~1us in,
            # then 256-col chunks through the rows the h1 phase reads; the
            # remainder loads after d1 so d1 lands sooner. The h1 phase
            # below iterates the same chunk list.
            hT_sb = consts.tile([P, KC, B], BF16)
            h1_rows = n1t * P
            # 256-col chunks (512B descriptors -- the no-penalty minimum)
            # covering the h1 rows plus whatever the warmup heads read.
            early_rows = min(max(-(-h1_rows // BH1) * BH1, 7 * P + BH1 - 1), B)
            early_rows = min(-(-early_rows // BH1) * BH1, B)
            bounds = list(range(0, early_rows + 1, BH1))
            h1_chunks = []
            for lo, hi in zip(bounds, bounds[1:]):
                if lo < h1_rows:
                    h1_chunks.append((lo, min(hi, h1_rows)))

            def load_hT(lo, hi):
                nc.sync.dma_start(hT_sb[:, :, lo:hi], hT3[:, :, lo:hi])

            # k-halves of down1 + the first hT chunk land first so the
            # first h1 matmuls (k-chunks 0-3) start ~2us earlier.
            kh = KC // 2
            if n1t:
                nc.sync.dma_start(down1_sb[:, :kh], dn3[:, :kh])
            nc.sync.dma_start(hT_sb[:, :kh, : bounds[1]], hT3[:, :kh, : bounds[1]])
            if n1t:
                nc.sync.dma_start(down1_sb[:, kh:], dn3[:, kh:])
            nc.sync.dma_start(hT_sb[:, kh:, : bounds[1]], hT3[:, kh:, : bounds[1]])
            for lo, hi in zip(bounds[1:], bounds[2:]):
                load_hT(lo, hi)
            wh_sb = consts.tile([P, KC, HEAD_C], BF16)
            nc.sync.dma_start(wh_sb[:], wh.rearrange("(ko p) v -> p ko v", p=P))
            d1_sb = consts.tile([P, KC1, T1_C], BF16)
            m0_sb = consts.tile([P, NBT], F32)
            m1_sb = consts.tile([P, NBT], F32)
            w0_sb = consts.tile([P, KC, T0_C], BF16)
            if n1t:
                nc.sync.dma_start(m1_sb[:], m1c[:, :])
                d13 = d1.rearrange("(ko p) v -> p ko v", p=P)
                half = T1_C // 2
                nc.sync.dma_start(d1_sb[:, :, :half], d13[:, :, :half])
                nc.sync.dma_start(d1_sb[:, :, half:], d13[:, :, half:])
            if t0hi > t0lo:
                nc.sync.dma_start(m0_sb[:], m0c[:, :])
            if bounds[-1] < B:
                load_hT(bounds[-1], B)
            if t0hi > t0lo:
                nc.sync.dma_start(w0_sb[:], w0.rearrange("(ko p) v -> p ko v", p=P))

            h1T_sb = consts.tile([P, KC1, B], BF16)

            # Phase 1: h1T[m, b] = sum_k down1[k, m] hT[k, b], only for the
            # batch chunks t1 tiles will read, chunk by chunk as hT lands.
            ev = 0  # round-robin eviction engine
            for c, (lo, hi) in enumerate(h1_chunks):
                w = hi - lo
                bsl = slice(lo, hi)
                for m in range(KC1):
                    ps = psum.tile([P, BH], F32, tag="ps", name=f"ps_h1_{c}_{m}")
                    for kc in range(KC):
                        nc.tensor.matmul(
                            ps[:, :w],
                            down1_sb[:, kc, m * P : (m + 1) * P],
                            hT_sb[:, kc, bsl],
                            start=(kc == 0),
                            stop=(kc == KC - 1),
                        )
                    dst = h1T_sb[:, m, bsl]
                    if ev == 0:
                        nc.scalar.copy(dst, ps[:, :w])
                    else:
                        nc.vector.tensor_copy(out=dst, in_=ps[:, :w])
                    ev = (ev + 1) % 2

            # Phase 2: per 128-row tile, whichever of {t0, head, t1} are
            # active into one staged slice, then a single DMA out. The
            # first few heads are emitted up front to cover the gap
            # between wh landing and d1 landing.
            stages = {}
            heads_done = set()

            def do_head(bt, stage):
                btsl = slice(bt * P, (bt + 1) * P)
                has_t0 = t0lo <= bt < t0hi
                m0s = m0_sb[:, bt : bt + 1]
                ps_h = psum.tile([P, BH], F32, tag="ps", name=f"ps_h_{bt}")
                if has_t0:
                    ps_a = psum.tile([P, BH], F32, tag="ps", name=f"ps_a_{bt}")
                    ps_b = psum.tile([P, BH], F32, tag="ps", name=f"ps_b_{bt}")
                for kc in range(KC):
                    first, last = kc == 0, kc == KC - 1
                    lhsT = hT_sb[:, kc, btsl]
                    nc.tensor.matmul(
                        ps_h[:, :HEAD_C], lhsT, wh_sb[:, kc, :],
                        start=first, stop=last,
                    )
                    if has_t0:
                        nc.tensor.matmul(
                            ps_a[:, :VT], lhsT, w0_sb[:, kc, 0:VT],
                            start=first, stop=last,
                        )
                        nc.tensor.matmul(
                            ps_b[:, :VT], lhsT, w0_sb[:, kc, VT : 2 * VT],
                            start=first, stop=last,
                        )
                nc.scalar.copy(stage[:, C_HEAD : C_HEAD + HEAD_C], ps_h[:, :HEAD_C])
                if has_t0:
                    nc.vector.tensor_scalar_mul(
                        out=stage[:, C_T0 : C_T0 + VT], in0=ps_a[:, :VT], scalar1=m0s
                    )
                    nc.scalar.mul(
                        stage[:, C_T0 + VT : C_T0 + 2 * VT], ps_b[:, :VT], m0s
                    )

            def get_stage(bt):
                if bt not in stages:
                    stages[bt] = opool.tile(
                        [P, OUT_C], BF16, tag="stage", name=f"stage_{bt}"
                    )
                return stages[bt]

            # Head warmup: fills the PE while d1 is still in flight.
            for bt in range(min(7, NBT)):
                do_head(bt, get_stage(bt))
                heads_done.add(bt)

            for bt in range(NBT):
                btsl = slice(bt * P, (bt + 1) * P)
                row0 = bt * P
                has_t1 = bt < n1t
                has_t0 = t0lo <= bt < t0hi
                stage = get_stage(bt)
                m1s = m1_sb[:, bt : bt + 1]

                # t1 in two waves of 5 v-tiles (5 psum banks in flight per
                # wave; stationary h1T block reused across the wave).
                if has_t1:
                    for w in range(2):
                        pss = [
                            psum.tile(
                                [P, BH], F32, tag="ps", name=f"ps_t1_{bt}_{w}_{i}"
                            )
                            for i in range(5)
                        ]
                        for kc in range(KC1):
                            first, last = kc == 0, kc == KC1 - 1
                            lhsT = h1T_sb[:, kc, btsl]
                            for i in range(5):
                                vt = w * 5 + i
                                nc.tensor.matmul(
                                    pss[i][:, :VT],
                                    lhsT,
                                    d1_sb[:, kc, vt * VT : (vt + 1) * VT],
                                    start=first,
                                    stop=last,
                                )
                        for i in range(5):
                            vt = w * 5 + i
                            c0 = C_T1 + vt * VT
                            dst = stage[:, c0 : c0 + VT]
                            src = pss[i][:, :VT]
                            if i < 3:
                                nc.vector.tensor_scalar_mul(
                                    out=dst, in0=src, scalar1=m1s
                                )
                            else:
                                nc.scalar.mul(dst, src, m1s)

                if bt not in heads_done:
                    do_head(bt, stage)

                # One DMA for the contiguous span of active segments (two
                # for the final tile, so the very last transfer is small).
                lo = C_T0 if has_t0 else C_HEAD
                hi = C_T1 + T1_C if has_t1 else C_T1
                if bt == NBT - 1 and lo < C_HEAD:
                    nc.sync.dma_start(
                        out[row0 : row0 + P, lo:C_HEAD], stage[:, lo:C_HEAD]
                    )
                    nc.sync.dma_start(
                        out[row0 : row0 + P, C_HEAD : C_HEAD + 128],
                        stage[:, C_HEAD : C_HEAD + 128],
                    )
                    nc.sync.dma_start(
                        out[row0 : row0 + P, C_HEAD + 128 : hi],
                        stage[:, C_HEAD + 128 : hi],
                    )
                else:
                    nc.sync.dma_start(
                        out[row0 : row0 + P, lo:hi], stage[:, lo:hi]
                    )

    nc.compile()
    return nc


def _get_compiled(n1t, t0lo, t0hi):
    key = (n1t, t0lo, t0hi)
    if key not in _compiled:
        _compiled[key] = _build(*key)
    return _compiled[key]


def _prep_inputs(hidden, target, head_w, down0, dec0, down1, dec1):
    f32 = np.float32
    hidden = np.asarray(hidden, dtype=f32)
    target = np.asarray(target)
    head_w = np.asarray(head_w, dtype=f32)
    down0 = np.asarray(down0, dtype=f32)
    dec0 = np.asarray(dec0, dtype=f32)
    down1 = np.asarray(down1, dtype=f32)
    dec1 = np.asarray(dec1, dtype=f32)

    # Cluster masks (cutoffs [2000, 10000, 50000]).
    m0 = ((target >= 2000) & (target < 10000)).astype(f32)
    m1 = ((target >= 10000) & (target < 50000)).astype(f32)
    # Permute rows: cluster-1 first, cluster-0 second, rest last, so each
    # tail decode touches a contiguous minimal range of 128-row tiles.
    key = np.where(m1 > 0, 0, np.where(m0 > 0, 1, 2))
    perm = np.argsort(key, kind="stable")
    n1 = int((m1 > 0).sum())
    n0 = int((m0 > 0).sum())
    n1t = -(-n1 // P)  # tiles 0..n1t-1 compute t1
    if n0:
        t0lo = n1 // P
        t0hi = -(-(n1 + n0) // P)  # tiles t0lo..t0hi-1 compute t0
    else:
        t0lo = t0hi = 0

    hidden = hidden[perm]
    m0 = m0[perm]
    m1 = m1[perm]

    hT = np.ascontiguousarray(hidden.T).astype(NP_BF16)
    whp = np.zeros((H, HEAD_PAD), dtype=f32)
    whp[:, :HEAD] = head_w
    whp = whp.astype(NP_BF16)
    # Fold the t0 branch: (h @ down0) @ dec0 == h @ (down0 @ dec0).
    w0eff = (down0 @ dec0).astype(NP_BF16)
    down1_b = down1.astype(NP_BF16)
    dec1_b = dec1.astype(NP_BF16)
    # Masks laid out [128, 16]: column bt holds tile bt's per-row scalars.
    m0c = np.ascontiguousarray(m0.reshape(NBT, P).T)
    m1c = np.ascontiguousarray(m1.reshape(NBT, P).T)

    in_maps = []
    for c in range(NCORES):
        in_maps.append(
            {
                "hT": hT,
                "wh": np.ascontiguousarray(whp[:, c * HEAD_C : (c + 1) * HEAD_C]),
                "w0": np.ascontiguousarray(w0eff[:, c * T0_C : (c + 1) * T0_C]),
                "down1": down1_b,
                "d1": np.ascontiguousarray(dec1_b[:, c * T1_C : (c + 1) * T1_C]),
                "m0c": m0c,
                "m1c": m1c,
            }
        )
    meta = {"perm": perm, "n1t": n1t, "t0lo": t0lo, "t0hi": t0hi}
    return in_maps, meta


def _assemble(results, meta):
    n1t, t0lo, t0hi = meta["n1t"], meta["t0lo"], meta["t0hi"]
    full = np.zeros((B, HEAD + T0 + T1), dtype=np.float32)
    r1 = n1t * P  # rows with t1 written
    r0lo, r0hi = t0lo * P, t0hi * P  # rows with t0 written
    for c in range(NCORES):
        o = np.asarray(results[c]["out"]).astype(np.float32)
        lo, hi = c * HEAD_C, (c + 1) * HEAD_C
        if lo < HEAD:
            full[:, lo : min(hi, HEAD)] = o[
                :, C_HEAD : C_HEAD + min(hi, HEAD) - lo
            ]
        full[r0lo:r0hi, HEAD + c * T0_C : HEAD + (c + 1) * T0_C] = o[
            r0lo:r0hi, C_T0 : C_T0 + T0_C
        ]
        full[:r1, HEAD + T0 + c * T1_C : HEAD + T0 + c * T1_C + T1_C] = o[
            :r1, C_T1 : C_T1 + T1_C
        ]
    # Undo the row permutation: permuted row i is original row perm[i].
    unperm = np.empty((B, full.shape[1]), dtype=full.dtype)
    unperm[meta["perm"]] = full
    return unperm


def run_on_device(inputs, trace=False, trace_cores=None):
    """Run the SPMD kernel; returns (full_output, BassKernelResults)."""
    in_maps, meta = _prep_inputs(**inputs)
    nc = _get_compiled(meta["n1t"], meta["t0lo"], meta["t0hi"])
    res = run_bass_kernel_spmd(
        nc,
        in_maps,
        list(range(NCORES)),
        trace=trace,
        trace_cores=trace_cores,
    )
    return _assemble(res.results, meta), res


def kernel(**inputs) -> np.ndarray:
    full, _ = run_on_device(inputs)
    return full


# revision 22
# speedup vs baseline: 1.0039x; 1.0039x over previous
"""Adaptive-softmax logits kernel for trn2 (8 NeuronCores, SPMD).

Problem: out = concat([hidden @ head_w,
                       ((hidden @ down0) @ dec0) * m0,
                       ((hidden @ down1) @ dec1) * m1], axis=1)
with hidden [2048, 1024], head_w [1024, 2002], dec0 [1024, 8000],
dec1 [256, 40000]; m0/m1 are per-row cluster masks from `target`.

Sharding: vocab-parallel. Each core gets 1/8 of every output segment
(head padded 2002->2048 so each core takes 256 head + 1000 t0 + 5000 t1
columns). Three host-side (algebraic, exact) restructurings cut the
device work:

1. The t0 branch is folded (W0 = down0 @ dec0, so t0 = hidden @ W0),
   removing the redundant 1024x1024 down-projection from every core.
2. Batch rows are permuted so cluster-1 rows come first, then
   cluster-0 rows, then the rest (row permutation commutes with every
   per-row op; the host inverse-permutes the assembled output). Each
   tail decode then only runs over the batch tiles its cluster
   occupies (~13/16 for t1, ~4/16 for t0 at the 80%/16% cluster
   rates); all other tail logits are exact zeros the host fills in.
   The on-device loop structure is compiled per (tile-range) tuple and
   cached, so any input distribution still produces a correct kernel.
3. Everything is bf16 (PSUM accumulates fp32): same 1 cycle/row PE
   rate as fp32r, half the HBM/SBUF bytes, absmax rel err ~3e-3 vs
   the 2e-2 gate. Output is written bf16 and upcast on the host.

On-device layout: out[b, v] tiles, batch on partitions; lhsT = x^T
k-chunk [128, 128] stationary, rhs = W [128, <=512] moving. Row masks
(per batch row = per partition) are applied during PSUM->SBUF
eviction as per-partition scalar multiplies, split across the
Activation and DVE engines (GPSIMD cannot read PSUM). Output columns
are ordered
[t0 | head | t1] so whatever subset of segments a 128-row tile
computes is one contiguous span -> a single dma_start per tile.
"""

import numpy as np
import ml_dtypes

import concourse.mybir as mybir
import concourse.tile as tile
from concourse import bacc
from concourse.bass_utils import run_bass_kernel_spmd

# Problem shapes (hardcoded per the grading contract).
B = 2048  # batch
H = 1024  # hidden
NCORES = 8
P = 128
KC = H // P  # 8 k-chunks for K=1024 contractions
HEAD = 2002
HEAD_PAD = 2048  # padded so each core gets 256 head columns
T0 = 8000  # cluster-0 decode width
T1 = 40000  # cluster-1 decode width
R1 = 256  # tail-1 down-projection width (down1 columns)
KC1 = R1 // P  # 2 k-chunks for the t1 decode contraction

# Per-core column counts.
HEAD_C = HEAD_PAD // NCORES  # 256
T0_C = T0 // NCORES  # 1000
T1_C = T1 // NCORES  # 5000
OUT_C = HEAD_C + T0_C + T1_C  # 6256
# On-device column layout: [t0 | head | t1] so any contiguous run of
# active segments is one DMA.
C_T0 = 0
C_HEAD = T0_C
C_T1 = T0_C + HEAD_C

NBT = B // P  # 16 batch tiles of 128 rows
VT = 500  # decode free-dim tile
T1_VT = T1_C // VT  # 10

BH = 512  # psum bank = 512 fp32
BH1 = 256  # h1-phase batch tile / hT DMA chunk
NBH1 = B // BH1  # 8

F32 = mybir.dt.float32
BF16 = mybir.dt.bfloat16

NP_BF16 = np.dtype(ml_dtypes.bfloat16)

_compiled = {}  # (n1t, t0lo, t0hi) -> nc


def _build(n1t, t0lo, t0hi):
    """n1t: # of 128-row tiles (from 0) computing t1; [t0lo, t0hi): tile
    range computing t0. All 16 tiles compute the head."""
    nc = bacc.Bacc(None)

    hT = nc.declare_dram_parameter("hT", [H, B], BF16, isOutput=False)
    wh = nc.declare_dram_parameter("wh", [H, HEAD_C], BF16, isOutput=False)
    w0 = nc.declare_dram_parameter("w0", [H, T0_C], BF16, isOutput=False)
    down1 = nc.declare_dram_parameter("down1", [H, R1], BF16, isOutput=False)
    d1 = nc.declare_dram_parameter("d1", [R1, T1_C], BF16, isOutput=False)
    m0c = nc.declare_dram_parameter("m0c", [P, NBT], F32, isOutput=False)
    m1c = nc.declare_dram_parameter("m1c", [P, NBT], F32, isOutput=False)
    out = nc.declare_dram_parameter("out", [B, OUT_C], BF16, isOutput=True)

    hT3 = hT.rearrange("(ko p) b -> p ko b", p=P)

    with tile.TileContext(nc) as tc:
        with (
            tc.tile_pool(name="consts", bufs=1) as consts,
            tc.tile_pool(name="opool", bufs=8) as opool,
            tc.tile_pool(name="psum", bufs=8, space="PSUM") as psum,
        ):
            # Input DMAs, in the order compute consumes them (DMA transfers
            # serialize, so this order sets when each tensor lands).
            down1_sb = consts.tile([P, KC, R1], BF16)
            dn3 = down1.rearrange("(ko p) m -> p ko m", p=P)
            # hT chunking: a small first chunk so the PE starts ~1us in,
            # then 256-col chunks through the rows the h1 phase reads; the
            # remainder loads after d1 so d1 lands sooner. The h1 phase
            # below iterates the same chunk list.
            hT_sb = consts.tile([P, KC, B], BF16)
            h1_rows = n1t * P
            # 256-col chunks (512B descriptors -- the no-penalty minimum)
            # covering the h1 rows plus whatever the warmup heads read.
            early_rows = min(max(-(-h1_rows // BH1) * BH1, 7 * P + BH1 - 1), B)
            early_rows = min(-(-early_rows // BH1) * BH1, B)
            bounds = list(range(0, early_rows + 1, BH1))
            h1_chunks = []
            for lo, hi in zip(bounds, bounds[1:]):
                if lo < h1_rows:
                    h1_chunks.append((lo, min(hi, h1_rows)))

            def load_hT(lo, hi, split=False):
                if split:
                    # k-halves: h1 matmuls for k-chunks 0-3 start while
                    # k-chunks 4-7 are still in flight.
                    nc.sync.dma_start(hT_sb[:, :kh, lo:hi], hT3[:, :kh, lo:hi])
                    nc.sync.dma_start(hT_sb[:, kh:, lo:hi], hT3[:, kh:, lo:hi])
                else:
                    nc.sync.dma_start(hT_sb[:, :, lo:hi], hT3[:, :, lo:hi])

            # k-halves of down1 + the first hT chunk land first so the
            # first h1 matmuls (k-chunks 0-3) start ~2us earlier.
            kh = KC // 2
            if n1t:
                nc.sync.dma_start(down1_sb[:, :kh], dn3[:, :kh])
            nc.sync.dma_start(hT_sb[:, :kh, : bounds[1]], hT3[:, :kh, : bounds[1]])
            if n1t:
                nc.sync.dma_start(down1_sb[:, kh:], dn3[:, kh:])
            nc.sync.dma_start(hT_sb[:, kh:, : bounds[1]], hT3[:, kh:, : bounds[1]])
            for lo, hi in zip(bounds[1:], bounds[2:]):
                load_hT(lo, hi)
            wh_sb = consts.tile([P, KC, HEAD_C], BF16)
            nc.sync.dma_start(wh_sb[:], wh.rearrange("(ko p) v -> p ko v", p=P))
            d1_sb = consts.tile([P, KC1, T1_C], BF16)
            m0_sb = consts.tile([P, NBT], F32)
            m1_sb = consts.tile([P, NBT], F32)
            w0_sb = consts.tile([P, KC, T0_C], BF16)
            if n1t:
                nc.sync.dma_start(m1_sb[:], m1c[:, :])
                d13 = d1.rearrange("(ko p) v -> p ko v", p=P)
                half = T1_C // 2
                nc.sync.dma_start(d1_sb[:, :, :half], d13[:, :, :half])
                nc.sync.dma_start(d1_sb[:, :, half:], d13[:, :, half:])
            if t0hi > t0lo:
                nc.sync.dma_start(m0_sb[:], m0c[:, :])
            if bounds[-1] < B:
                load_hT(bounds[-1], B)
            if t0hi > t0lo:
                nc.sync.dma_start(w0_sb[:], w0.rearrange("(ko p) v -> p ko v", p=P))

            h1T_sb = consts.tile([P, KC1, B], BF16)

            # Phase 1: h1T[m, b] = sum_k down1[k, m] hT[k, b], only for the
            # batch chunks t1 tiles will read, chunk by chunk as hT lands.
            ev = 0  # round-robin eviction engine
            for c, (lo, hi) in enumerate(h1_chunks):
                w = hi - lo
                bsl = slice(lo, hi)
                for m in range(KC1):
                    ps = psum.tile([P, BH], F32, tag="ps", name=f"ps_h1_{c}_{m}")
                    for kc in range(KC):
                        nc.tensor.matmul(
                            ps[:, :w],
                            down1_sb[:, kc, m * P : (m + 1) * P],
                            hT_sb[:, kc, bsl],
                            start=(kc == 0),
                            stop=(kc == KC - 1),
                        )
                    dst = h1T_sb[:, m, bsl]
                    if ev == 0:
                        nc.scalar.copy(dst, ps[:, :w])
                    else:
                        nc.vector.tensor_copy(out=dst, in_=ps[:, :w])
                    ev = (ev + 1) % 2

            # Phase 2: per 128-row tile, whichever of {t0, head, t1} are
            # active into one staged slice, then a single DMA out. The
            # first few heads are emitted up front to cover the gap
            # between wh landing and d1 landing.
            stages = {}
            heads_done = set()

            def do_head(bt, stage):
                btsl = slice(bt * P, (bt + 1) * P)
                has_t0 = t0lo <= bt < t0hi
                m0s = m0_sb[:, bt : bt + 1]
                ps_h = psum.tile([P, BH], F32, tag="ps", name=f"ps_h_{bt}")
                if has_t0:
                    ps_a = psum.tile([P, BH], F32, tag="ps", name=f"ps_a_{bt}")
                    ps_b = psum.tile([P, BH], F32, tag="ps", name=f"ps_b_{bt}")
                for kc in range(KC):
                    first, last = kc == 0, kc == KC - 1
                    lhsT = hT_sb[:, kc, btsl]
                    nc.tensor.matmul(
                        ps_h[:, :HEAD_C], lhsT, wh_sb[:, kc, :],
                        start=first, stop=last,
                    )
                    if has_t0:
                        nc.tensor.matmul(
                            ps_a[:, :VT], lhsT, w0_sb[:, kc, 0:VT],
                            start=first, stop=last,
                        )
                        nc.tensor.matmul(
                            ps_b[:, :VT], lhsT, w0_sb[:, kc, VT : 2 * VT],
                            start=first, stop=last,
                        )
                nc.scalar.copy(stage[:, C_HEAD : C_HEAD + HEAD_C], ps_h[:, :HEAD_C])
                if has_t0:
                    nc.vector.tensor_scalar_mul(
                        out=stage[:, C_T0 : C_T0 + VT], in0=ps_a[:, :VT], scalar1=m0s
                    )
                    nc.scalar.mul(
                        stage[:, C_T0 + VT : C_T0 + 2 * VT], ps_b[:, :VT], m0s
                    )

            def get_stage(bt):
                if bt not in stages:
                    stages[bt] = opool.tile(
                        [P, OUT_C], BF16, tag="stage", name=f"stage_{bt}"
                    )
                return stages[bt]

            # Head warmup: fills the PE while d1 is still in flight.
            for bt in range(min(7, NBT)):
                do_head(bt, get_stage(bt))
                heads_done.add(bt)

            for bt in range(NBT):
                btsl = slice(bt * P, (bt + 1) * P)
                row0 = bt * P
                has_t1 = bt < n1t
                has_t0 = t0lo <= bt < t0hi
                stage = get_stage(bt)
                m1s = m1_sb[:, bt : bt + 1]

                # t1 in two waves of 5 v-tiles (5 psum banks in flight per
                # wave; stationary h1T block reused across the wave).
                if has_t1:
                    for w in range(2):
                        pss = [
                            psum.tile(
                                [P, BH], F32, tag="ps", name=f"ps_t1_{bt}_{w}_{i}"
                            )
                            for i in range(5)
                        ]
                        for kc in range(KC1):
                            first, last = kc == 0, kc == KC1 - 1
                            lhsT = h1T_sb[:, kc, btsl]
                            for i in range(5):
                                vt = w * 5 + i
                                nc.tensor.matmul(
                                    pss[i][:, :VT],
                                    lhsT,
                                    d1_sb[:, kc, vt * VT : (vt + 1) * VT],
                                    start=first,
                                    stop=last,
                                )
                        for i in range(5):
                            vt = w * 5 + i
                            c0 = C_T1 + vt * VT
                            dst = stage[:, c0 : c0 + VT]
                            src = pss[i][:, :VT]
                            if i < 3:
                                nc.vector.tensor_scalar_mul(
                                    out=dst, in0=src, scalar1=m1s
                                )
                            else:
                                nc.scalar.mul(dst, src, m1s)

                if bt not in heads_done:
                    do_head(bt, stage)

                # One DMA for the contiguous span of active segments (two
                # for the final tile, so the very last transfer is small).
                lo = C_T0 if has_t0 else C_HEAD
                hi = C_T1 + T1_C if has_t1 else C_T1
                if bt == NBT - 1 and lo < C_HEAD:
                    nc.sync.dma_start(
                        out[row0 : row0 + P, lo:C_HEAD], stage[:, lo:C_HEAD]
                    )
                    nc.sync.dma_start(
                        out[row0 : row0 + P, C_HEAD:hi], stage[:, C_HEAD:hi]
                    )
                else:
                    nc.sync.dma_start(
                        out[row0 : row0 + P, lo:hi], stage[:, lo:hi]
                    )

    nc.compile()
    return nc


def _get_compiled(n1t, t0lo, t0hi):
    key = (n1t, t0lo, t0hi)
    if key not in _compiled:
        _compiled[key] = _build(*key)
    return _compiled[key]


def _prep_inputs(hidden, target, head_w, down0, dec0, down1, dec1):
    f32 = np.float32
    hidden = np.asarray(hidden, dtype=f32)
    target = np.asarray(target)
    head_w = np.asarray(head_w, dtype=f32)
    down0 = np.asarray(down0, dtype=f32)
    dec0 = np.asarray(dec0, dtype=f32)
    down1 = np.asarray(down1, dtype=f32)
    dec1 = np.asarray(dec1, dtype=f32)

    # Cluster masks (cutoffs [2000, 10000, 50000]).
    m0 = ((target >= 2000) & (target < 10000)).astype(f32)
    m1 = ((target >= 10000) & (target < 50000)).astype(f32)
    # Permute rows: cluster-1 first, cluster-0 second, rest last, so each
    # tail decode touches a contiguous minimal range of 128-row tiles.
    key = np.where(m1 > 0, 0, np.where(m0 > 0, 1, 2))
    perm = np.argsort(key, kind="stable")
    n1 = int((m1 > 0).sum())
    n0 = int((m0 > 0).sum())
    n1t = -(-n1 // P)  # tiles 0..n1t-1 compute t1
    if n0:
        t0lo = n1 // P
        t0hi = -(-(n1 + n0) // P)  # tiles t0lo..t0hi-1 compute t0
    else:
        t0lo = t0hi = 0

    hidden = hidden[perm]
    m0 = m0[perm]
    m1 = m1[perm]

    hT = np.ascontiguousarray(hidden.T).astype(NP_BF16)
    whp = np.zeros((H, HEAD_PAD), dtype=f32)
    whp[:, :HEAD] = head_w
    whp = whp.astype(NP_BF16)
    # Fold the t0 branch: (h @ down0) @ dec0 == h @ (down0 @ dec0).
    w0eff = (down0 @ dec0).astype(NP_BF16)
    down1_b = down1.astype(NP_BF16)
    dec1_b = dec1.astype(NP_BF16)
    # Masks laid out [128, 16]: column bt holds tile bt's per-row scalars.
    m0c = np.ascontiguousarray(m0.reshape(NBT, P).T)
    m1c = np.ascontiguousarray(m1.reshape(NBT, P).T)

    in_maps = []
    for c in range(NCORES):
        in_maps.append(
            {
                "hT": hT,
                "wh": np.ascontiguousarray(whp[:, c * HEAD_C : (c + 1) * HEAD_C]),
                "w0": np.ascontiguousarray(w0eff[:, c * T0_C : (c + 1) * T0_C]),
                "down1": down1_b,
                "d1": np.ascontiguousarray(dec1_b[:, c * T1_C : (c + 1) * T1_C]),
                "m0c": m0c,
                "m1c": m1c,
            }
        )
    meta = {"perm": perm, "n1t": n1t, "t0lo": t0lo, "t0hi": t0hi}
    return in_maps, meta


def _assemble(results, meta):
    n1t, t0lo, t0hi = meta["n1t"], meta["t0lo"], meta["t0hi"]
    full = np.zeros((B, HEAD + T0 + T1), dtype=np.float32)
    r1 = n1t * P  # rows with t1 written
    r0lo, r0hi = t0lo * P, t0hi * P  # rows with t0 written
    for c in range(NCORES):
        o = np.asarray(results[c]["out"]).astype(np.float32)
        lo, hi = c * HEAD_C, (c + 1) * HEAD_C
        if lo < HEAD:
            full[:, lo : min(hi, HEAD)] = o[
                :, C_HEAD : C_HEAD + min(hi, HEAD) - lo
            ]
        full[r0lo:r0hi, HEAD + c * T0_C : HEAD + (c + 1) * T0_C] = o[
            r0lo:r0hi, C_T0 : C_T0 + T0_C
        ]
        full[:r1, HEAD + T0 + c * T1_C : HEAD + T0 + c * T1_C + T1_C] = o[
            :r1, C_T1 : C_T1 + T1_C
        ]
    # Undo the row permutation: permuted row i is original row perm[i].
    unperm = np.empty((B, full.shape[1]), dtype=full.dtype)
    unperm[meta["perm"]] = full
    return unperm


def run_on_device(inputs, trace=False, trace_cores=None):
    """Run the SPMD kernel; returns (full_output, BassKernelResults)."""
    in_maps, meta = _prep_inputs(**inputs)
    nc = _get_compiled(meta["n1t"], meta["t0lo"], meta["t0hi"])
    res = run_bass_kernel_spmd(
        nc,
        in_maps,
        list(range(NCORES)),
        trace=trace,
        trace_cores=trace_cores,
    )
    return _assemble(res.results, meta), res


def kernel(**inputs) -> np.ndarray:
    full, _ = run_on_device(inputs)
    return full


# revision 23
# speedup vs baseline: 1.0073x; 1.0034x over previous
"""Adaptive-softmax logits kernel for trn2 (8 NeuronCores, SPMD).

Problem: out = concat([hidden @ head_w,
                       ((hidden @ down0) @ dec0) * m0,
                       ((hidden @ down1) @ dec1) * m1], axis=1)
with hidden [2048, 1024], head_w [1024, 2002], dec0 [1024, 8000],
dec1 [256, 40000]; m0/m1 are per-row cluster masks from `target`.

Sharding: vocab-parallel. Each core gets 1/8 of every output segment
(head padded 2002->2048 so each core takes 256 head + 1000 t0 + 5000 t1
columns). Three host-side (algebraic, exact) restructurings cut the
device work:

1. The t0 branch is folded (W0 = down0 @ dec0, so t0 = hidden @ W0),
   removing the redundant 1024x1024 down-projection from every core.
2. Batch rows are permuted so cluster-1 rows come first, then
   cluster-0 rows, then the rest (row permutation commutes with every
   per-row op; the host inverse-permutes the assembled output). Each
   tail decode then only runs over the batch tiles its cluster
   occupies (~13/16 for t1, ~4/16 for t0 at the 80%/16% cluster
   rates); all other tail logits are exact zeros the host fills in.
   The on-device loop structure is compiled per (tile-range) tuple and
   cached, so any input distribution still produces a correct kernel.
3. Everything is bf16 (PSUM accumulates fp32): same 1 cycle/row PE
   rate as fp32r, half the HBM/SBUF bytes, absmax rel err ~3e-3 vs
   the 2e-2 gate. Output is written bf16 and upcast on the host.

On-device layout: out[b, v] tiles, batch on partitions; lhsT = x^T
k-chunk [128, 128] stationary, rhs = W [128, <=512] moving. Row masks
(per batch row = per partition) are applied during PSUM->SBUF
eviction as per-partition scalar multiplies, split across the
Activation and DVE engines (GPSIMD cannot read PSUM). Output columns
are ordered
[t0 | head | t1] so whatever subset of segments a 128-row tile
computes is one contiguous span -> a single dma_start per tile.
"""

import numpy as np
import ml_dtypes

import concourse.mybir as mybir
import concourse.tile as tile
from concourse import bacc
from concourse.bass_utils import run_bass_kernel_spmd

# Problem shapes (hardcoded per the grading contract).
B = 2048  # batch
H = 1024  # hidden
NCORES = 8
P = 128
KC = H // P  # 8 k-chunks for K=1024 contractions
HEAD = 2002
HEAD_PAD = 2048  # padded so each core gets 256 head columns
T0 = 8000  # cluster-0 decode width
T1 = 40000  # cluster-1 decode width
R1 = 256  # tail-1 down-projection width (down1 columns)
KC1 = R1 // P  # 2 k-chunks for the t1 decode contraction

# Per-core column counts.
HEAD_C = HEAD_PAD // NCORES  # 256
T0_C = T0 // NCORES  # 1000
T1_C = T1 // NCORES  # 5000
OUT_C = HEAD_C + T0_C + T1_C  # 6256
# On-device column layout: [t0 | head | t1] so any contiguous run of
# active segments is one DMA.
C_T0 = 0
C_HEAD = T0_C
C_T1 = T0_C + HEAD_C

NBT = B // P  # 16 batch tiles of 128 rows
VT = 500  # decode free-dim tile
T1_VT = T1_C // VT  # 10

BH = 512  # psum bank = 512 fp32
BH1 = 256  # h1-phase batch tile / hT DMA chunk
NBH1 = B // BH1  # 8

F32 = mybir.dt.float32
BF16 = mybir.dt.bfloat16

NP_BF16 = np.dtype(ml_dtypes.bfloat16)

_compiled = {}  # (n1t, t0lo, t0hi) -> nc


def _build(n1t, t0lo, t0hi):
    """n1t: # of 128-row tiles (from 0) computing t1; [t0lo, t0hi): tile
    range computing t0. All 16 tiles compute the head."""
    nc = bacc.Bacc(None)

    hT = nc.declare_dram_parameter("hT", [H, B], BF16, isOutput=False)
    wh = nc.declare_dram_parameter("wh", [H, HEAD_C], BF16, isOutput=False)
    w0 = nc.declare_dram_parameter("w0", [H, T0_C], BF16, isOutput=False)
    down1 = nc.declare_dram_parameter("down1", [H, R1], BF16, isOutput=False)
    d1 = nc.declare_dram_parameter("d1", [R1, T1_C], BF16, isOutput=False)
    m0c = nc.declare_dram_parameter("m0c", [P, NBT], F32, isOutput=False)
    m1c = nc.declare_dram_parameter("m1c", [P, NBT], F32, isOutput=False)
    out = nc.declare_dram_parameter("out", [B, OUT_C], BF16, isOutput=True)

    hT3 = hT.rearrange("(ko p) b -> p ko b", p=P)

    with tile.TileContext(nc) as tc:
        with (
            tc.tile_pool(name="consts", bufs=1) as consts,
            tc.tile_pool(name="opool", bufs=4) as opool,
            tc.tile_pool(name="psum", bufs=8, space="PSUM") as psum,
        ):
            # Input DMAs, in the order compute consumes them (DMA transfers
            # serialize, so this order sets when each tensor lands).
            down1_sb = consts.tile([P, KC, R1], BF16)
            dn3 = down1.rearrange("(ko p) m -> p ko m", p=P)
            # hT chunking: a small first chunk so the PE starts ~1us in,
            # then 256-col chunks through the rows the h1 phase reads; the
            # remainder loads after d1 so d1 lands sooner. The h1 phase
            # below iterates the same chunk list.
            hT_sb = consts.tile([P, KC, B], BF16)
            h1_rows = n1t * P
            # 256-col chunks (512B descriptors -- the no-penalty minimum)
            # covering the h1 rows plus whatever the warmup heads read.
            early_rows = min(max(-(-h1_rows // BH1) * BH1, 7 * P + BH1 - 1), B)
            early_rows = min(-(-early_rows // BH1) * BH1, B)
            bounds = list(range(0, early_rows + 1, BH1))
            h1_chunks = []
            for lo, hi in zip(bounds, bounds[1:]):
                if lo < h1_rows:
                    h1_chunks.append((lo, min(hi, h1_rows)))

            def load_hT(lo, hi, split=False):
                if split:
                    # k-halves: h1 matmuls for k-chunks 0-3 start while
                    # k-chunks 4-7 are still in flight.
                    nc.sync.dma_start(hT_sb[:, :kh, lo:hi], hT3[:, :kh, lo:hi])
                    nc.sync.dma_start(hT_sb[:, kh:, lo:hi], hT3[:, kh:, lo:hi])
                else:
                    nc.sync.dma_start(hT_sb[:, :, lo:hi], hT3[:, :, lo:hi])

            # k-halves of down1 + the first hT chunk land first so the
            # first h1 matmuls (k-chunks 0-3) start ~2us earlier.
            kh = KC // 2
            if n1t:
                nc.sync.dma_start(down1_sb[:, :kh], dn3[:, :kh])
            nc.sync.dma_start(hT_sb[:, :kh, : bounds[1]], hT3[:, :kh, : bounds[1]])
            if n1t:
                nc.sync.dma_start(down1_sb[:, kh:], dn3[:, kh:])
            nc.sync.dma_start(hT_sb[:, kh:, : bounds[1]], hT3[:, kh:, : bounds[1]])
            wh_sb = consts.tile([P, KC, HEAD_C], BF16)
            nc.sync.dma_start(wh_sb[:], wh.rearrange("(ko p) v -> p ko v", p=P))
            for lo, hi in zip(bounds[1:], bounds[2:]):
                load_hT(lo, hi)
            d1_sb = consts.tile([P, KC1, T1_C], BF16)
            m0_sb = consts.tile([P, NBT], F32)
            m1_sb = consts.tile([P, NBT], F32)
            w0_sb = consts.tile([P, KC, T0_C], BF16)
            if n1t:
                nc.sync.dma_start(m1_sb[:], m1c[:, :])
                d13 = d1.rearrange("(ko p) v -> p ko v", p=P)
                half = T1_C // 2
                nc.sync.dma_start(d1_sb[:, :, :half], d13[:, :, :half])
                nc.sync.dma_start(d1_sb[:, :, half:], d13[:, :, half:])
            if t0hi > t0lo:
                nc.sync.dma_start(m0_sb[:], m0c[:, :])
            if bounds[-1] < B:
                load_hT(bounds[-1], B)
            if t0hi > t0lo:
                nc.sync.dma_start(w0_sb[:], w0.rearrange("(ko p) v -> p ko v", p=P))

            h1T_sb = consts.tile([P, KC1, B], BF16)
            head_st = consts.tile([P, NBT, HEAD_C], BF16)

            heads_done = set()

            def do_head(bt):
                btsl = slice(bt * P, (bt + 1) * P)
                ps_h = psum.tile([P, BH], F32, tag="ps", name=f"ps_h_{bt}")
                for kc in range(KC):
                    nc.tensor.matmul(
                        ps_h[:, :HEAD_C],
                        hT_sb[:, kc, btsl],
                        wh_sb[:, kc, :],
                        start=(kc == 0),
                        stop=(kc == KC - 1),
                    )
                nc.scalar.copy(head_st[:, bt], ps_h[:, :HEAD_C])
                heads_done.add(bt)

            # Phase 1: h1T chunks interleaved with head tiles as hT lands,
            # so the PE has ~2x the work per arriving chunk and never
            # starves while inputs stream in. Heads go to their own SBUF
            # slab (no per-tile stage held open).
            nxt_head = 0
            for c, (lo, hi) in enumerate(h1_chunks):
                w = hi - lo
                bsl = slice(lo, hi)
                for m in range(KC1):
                    ps = psum.tile([P, BH], F32, tag="ps", name=f"ps_h1_{c}_{m}")
                    for kc in range(KC):
                        nc.tensor.matmul(
                            ps[:, :w],
                            down1_sb[:, kc, m * P : (m + 1) * P],
                            hT_sb[:, kc, bsl],
                            start=(kc == 0),
                            stop=(kc == KC - 1),
                        )
                    nc.vector.tensor_copy(out=h1T_sb[:, m, bsl], in_=ps[:, :w])
                while (nxt_head + 1) * P <= bounds[c + 1] and nxt_head < NBT:
                    do_head(nxt_head)
                    nxt_head += 1
            while (nxt_head + 1) * P <= bounds[-1] and nxt_head < NBT:
                do_head(nxt_head)
                nxt_head += 1

            # Phase 2: per 128-row tile, t1/t0 into a staged slice; heads
            # (already in head_st) and the tail segments each leave in
            # their own DMA.
            stages = {}

            def get_stage(bt):
                if bt not in stages:
                    stages[bt] = opool.tile(
                        [P, OUT_C], BF16, tag="stage", name=f"stage_{bt}"
                    )
                return stages[bt]

            for bt in range(NBT):
                btsl = slice(bt * P, (bt + 1) * P)
                row0 = bt * P
                has_t1 = bt < n1t
                has_t0 = t0lo <= bt < t0hi
                stage = get_stage(bt) if (has_t1 or has_t0) else None
                m1s = m1_sb[:, bt : bt + 1]
                m0s = m0_sb[:, bt : bt + 1]

                # t1 in two waves of 5 v-tiles (5 psum banks in flight per
                # wave; stationary h1T block reused across the wave).
                if has_t1:
                    for w in range(2):
                        pss = [
                            psum.tile(
                                [P, BH], F32, tag="ps", name=f"ps_t1_{bt}_{w}_{i}"
                            )
                            for i in range(5)
                        ]
                        for kc in range(KC1):
                            first, last = kc == 0, kc == KC1 - 1
                            lhsT = h1T_sb[:, kc, btsl]
                            for i in range(5):
                                vt = w * 5 + i
                                nc.tensor.matmul(
                                    pss[i][:, :VT],
                                    lhsT,
                                    d1_sb[:, kc, vt * VT : (vt + 1) * VT],
                                    start=first,
                                    stop=last,
                                )
                        for i in range(5):
                            vt = w * 5 + i
                            c0 = C_T1 + vt * VT
                            dst = stage[:, c0 : c0 + VT]
                            src = pss[i][:, :VT]
                            if i < 3:
                                nc.vector.tensor_scalar_mul(
                                    out=dst, in0=src, scalar1=m1s
                                )
                            else:
                                nc.scalar.mul(dst, src, m1s)

                if has_t0:
                    ps_a = psum.tile([P, BH], F32, tag="ps", name=f"ps_a_{bt}")
                    ps_b = psum.tile([P, BH], F32, tag="ps", name=f"ps_b_{bt}")
                    for kc in range(KC):
                        first, last = kc == 0, kc == KC - 1
                        lhsT = hT_sb[:, kc, btsl]
                        nc.tensor.matmul(
                            ps_a[:, :VT], lhsT, w0_sb[:, kc, 0:VT],
                            start=first, stop=last,
                        )
                        nc.tensor.matmul(
                            ps_b[:, :VT], lhsT, w0_sb[:, kc, VT : 2 * VT],
                            start=first, stop=last,
                        )
                    nc.vector.tensor_scalar_mul(
                        out=stage[:, C_T0 : C_T0 + VT], in0=ps_a[:, :VT], scalar1=m0s
                    )
                    nc.scalar.mul(
                        stage[:, C_T0 + VT : C_T0 + 2 * VT], ps_b[:, :VT], m0s
                    )
                if bt not in heads_done:
                    do_head(bt)

                nc.sync.dma_start(
                    out[row0 : row0 + P, C_HEAD : C_HEAD + HEAD_C],
                    head_st[:, bt],
                )
                if has_t0:
                    nc.sync.dma_start(
                        out[row0 : row0 + P, C_T0:C_HEAD], stage[:, C_T0:C_HEAD]
                    )
                if has_t1:
                    nc.sync.dma_start(
                        out[row0 : row0 + P, C_T1 : C_T1 + T1_C],
                        stage[:, C_T1 : C_T1 + T1_C],
                    )

    nc.compile()
    return nc


def _get_compiled(n1t, t0lo, t0hi):
    key = (n1t, t0lo, t0hi)
    if key not in _compiled:
        _compiled[key] = _build(*key)
    return _compiled[key]


def _prep_inputs(hidden, target, head_w, down0, dec0, down1, dec1):
    f32 = np.float32
    hidden = np.asarray(hidden, dtype=f32)
    target = np.asarray(target)
    head_w = np.asarray(head_w, dtype=f32)
    down0 = np.asarray(down0, dtype=f32)
    dec0 = np.asarray(dec0, dtype=f32)
    down1 = np.asarray(down1, dtype=f32)
    dec1 = np.asarray(dec1, dtype=f32)

    # Cluster masks (cutoffs [2000, 10000, 50000]).
    m0 = ((target >= 2000) & (target < 10000)).astype(f32)
    m1 = ((target >= 10000) & (target < 50000)).astype(f32)
    # Permute rows: cluster-1 first, cluster-0 second, rest last, so each
    # tail decode touches a contiguous minimal range of 128-row tiles.
    key = np.where(m1 > 0, 0, np.where(m0 > 0, 1, 2))
    perm = np.argsort(key, kind="stable")
    n1 = int((m1 > 0).sum())
    n0 = int((m0 > 0).sum())
    n1t = -(-n1 // P)  # tiles 0..n1t-1 compute t1
    if n0:
        t0lo = n1 // P
        t0hi = -(-(n1 + n0) // P)  # tiles t0lo..t0hi-1 compute t0
    else:
        t0lo = t0hi = 0

    hidden = hidden[perm]
    m0 = m0[perm]
    m1 = m1[perm]

    hT = np.ascontiguousarray(hidden.T).astype(NP_BF16)
    whp = np.zeros((H, HEAD_PAD), dtype=f32)
    whp[:, :HEAD] = head_w
    whp = whp.astype(NP_BF16)
    # Fold the t0 branch: (h @ down0) @ dec0 == h @ (down0 @ dec0).
    w0eff = (down0 @ dec0).astype(NP_BF16)
    down1_b = down1.astype(NP_BF16)
    dec1_b = dec1.astype(NP_BF16)
    # Masks laid out [128, 16]: column bt holds tile bt's per-row scalars.
    m0c = np.ascontiguousarray(m0.reshape(NBT, P).T)
    m1c = np.ascontiguousarray(m1.reshape(NBT, P).T)

    in_maps = []
    for c in range(NCORES):
        in_maps.append(
            {
                "hT": hT,
                "wh": np.ascontiguousarray(whp[:, c * HEAD_C : (c + 1) * HEAD_C]),
                "w0": np.ascontiguousarray(w0eff[:, c * T0_C : (c + 1) * T0_C]),
                "down1": down1_b,
                "d1": np.ascontiguousarray(dec1_b[:, c * T1_C : (c + 1) * T1_C]),
                "m0c": m0c,
                "m1c": m1c,
            }
        )
    meta = {"perm": perm, "n1t": n1t, "t0lo": t0lo, "t0hi": t0hi}
    return in_maps, meta


def _assemble(results, meta):
    n1t, t0lo, t0hi = meta["n1t"], meta["t0lo"], meta["t0hi"]
    full = np.zeros((B, HEAD + T0 + T1), dtype=np.float32)
    r1 = n1t * P  # rows with t1 written
    r0lo, r0hi = t0lo * P, t0hi * P  # rows with t0 written
    for c in range(NCORES):
        o = np.asarray(results[c]["out"]).astype(np.float32)
        lo, hi = c * HEAD_C, (c + 1) * HEAD_C
        if lo < HEAD:
            full[:, lo : min(hi, HEAD)] = o[
                :, C_HEAD : C_HEAD + min(hi, HEAD) - lo
            ]
        full[r0lo:r0hi, HEAD + c * T0_C : HEAD + (c + 1) * T0_C] = o[
            r0lo:r0hi, C_T0 : C_T0 + T0_C
        ]
        full[:r1, HEAD + T0 + c * T1_C : HEAD + T0 + c * T1_C + T1_C] = o[
            :r1, C_T1 : C_T1 + T1_C
        ]
    # Undo the row permutation: permuted row i is original row perm[i].
    unperm = np.empty((B, full.shape[1]), dtype=full.dtype)
    unperm[meta["perm"]] = full
    return unperm


def run_on_device(inputs, trace=False, trace_cores=None):
    """Run the SPMD kernel; returns (full_output, BassKernelResults)."""
    in_maps, meta = _prep_inputs(**inputs)
    nc = _get_compiled(meta["n1t"], meta["t0lo"], meta["t0hi"])
    res = run_bass_kernel_spmd(
        nc,
        in_maps,
        list(range(NCORES)),
        trace=trace,
        trace_cores=trace_cores,
    )
    return _assemble(res.results, meta), res


def kernel(**inputs) -> np.ndarray:
    full, _ = run_on_device(inputs)
    return full
